# revision 6
# baseline (speedup 1.0000x reference)
"""NeighborConsistencyLoss on 8 Trainium2 NeuronCores.

Math:  loss = mean_s(1 - mean_k cos(z[s], z[knn[s,k]]))
            = 1 - (1/(S*K)) * sum_{s,k} u(z[s]) . u(z[knn[s,k]])
where u(x) = x/|x| (eps in max(|a||b|, eps) never binds for randn data).

Sharding: replicate z (pre-cast to bf16 on host), shard the S=1000
sampled centers across 8 cores (125 each, padded to 128).

Per-core device pipeline (v2 — dma_gather based):
 - z lives in HBM as bf16 [N, D] (host-side cast; halves gather traffic
   and feeds the PE directly). The 125*32 = 4000 neighbor rows are
   gathered by 7 `dma_gather` (InstDMAGatherAnt) instructions, one per
   32768-row bank of z (int16 local indices = row - 32768*b). One
   gather instruction carries ~768 descriptors at ~0.34ns/desc of
   SWDGE time vs 994ns+128desc for classic indirect1d — this removes
   the GpSimd descriptor-generation serial bottleneck that dominated
   v1 (33 indirect1d = 48us serial).
 - Banks are capped at 768 slots (6 batches of 128) for banks 0-5 and
   128 for bank 6 (rows 196608..199999); unused slots are padded with
   a duplicate valid index (gathers real data, masked out later) so no
   NaN/garbage can reach the matmuls. Gather layout: descriptor i ->
   dst[i%128, i//128, :].
 - Per 128-row batch: ssq via ACT Square+accum / DVE mult+accum
   (alternating), rnorm = 1/sqrt batched per 4 batches, then
   mask_rn = host_mask * rnorm (one [128,128] op, alternating engines)
   where host_mask[p, s] = 1 iff slot p of the batch belongs to center
   s. The PE accumulates V[s,:] += mask_rn^T @ raw_bf16_rows, folding
   both the neighbor-sum AND the per-row normalization into the
   matmul (no per-row scale pass over [128,512] data).
 - Centers: one classic indirect1d gather (128 rows, int32), norm
   computed early, r[s] = rnorm_c[s] * sum_d c[s,d]*V[s,d] (fused DVE
   op with accum), partial = cmask^T @ r via a tiny matmul.
 - Host combines: loss = 1 - total/(S*K).
"""

import numpy as np

N, D, K, S = 200000, 512, 32, 1000
NCORES = 8
SPC = S // NCORES            # 125 samples per core
P = 128
BANK = 32768                 # int16-addressable rows per dma_gather bank
NBANKS = 7                   # ceil(N / BANK)
CAPS = [768] * 6 + [128]     # slots per bank (multiple of 128)
NBAT = sum(CAPS) // P        # 37 neighbor batches
IDXCOLS = sum(CAPS) // 16    # 296 int16 columns
GRP = 8                      # batches per rnorm group

_cache = {}


def _build_module():
    import concourse.bacc as bacc
    import concourse.bass as bass
    import concourse.mybir as mybir
    import concourse.tile as tile

    f32 = mybir.dt.float32
    bf16 = mybir.dt.bfloat16
    i32 = mybir.dt.int32
    i16 = mybir.dt.int16
    AF = mybir.ActivationFunctionType
    ALU = mybir.AluOpType

    nc = bacc.Bacc(None, target_bir_lowering=False,
                   dynamic_dma_scratch_size=2**16, num_swdge_queues=4)
    z_t = nc.dram_tensor("z", [N, D], bf16, kind="ExternalInput")
    idx16_t = nc.dram_tensor("idx16", [P, IDXCOLS], i16, kind="ExternalInput")
    mask_t = nc.dram_tensor("nmask", [P, NBAT * P], bf16, kind="ExternalInput")
    cidx_t = nc.dram_tensor("cidx", [P, 1], i32, kind="ExternalInput")
    cmask_t = nc.dram_tensor("cmask", [P, 1], f32, kind="ExternalInput")
    out_t = nc.dram_tensor("out", [1, 1], f32, kind="ExternalOutput")

    with tile.TileContext(nc) as tc:
        with (
            tc.tile_pool(name="const", bufs=1) as const,
            tc.tile_pool(name="gath", bufs=1) as gath,
            tc.tile_pool(name="scr", bufs=2) as scr,
            tc.tile_pool(name="mrn", bufs=4) as mrn,
            tc.tile_pool(name="ps", bufs=1, space="PSUM") as ps,
        ):
            cidx_sb = const.tile([P, 1], i32, tag="cidx")
            nc.sync.dma_start(cidx_sb[:], cidx_t[:])
            idx16_sb = const.tile([P, IDXCOLS], i16, tag="idx16")
            nc.sync.dma_start(idx16_sb[:], idx16_t[:])

            # warm both activation tables during the dead startup window so
            # the 1.28us ACT_TABLE_LOADs don't stall the pipeline mid-stream
            warm = const.tile([P, 1], f32, tag="warm")
            warm2 = const.tile([P, 1], f32, tag="warm2")
            nc.vector.memset(warm[:], 1.0)
            nc.scalar.activation(warm2[:], warm[:], AF.Square)
            nc.scalar.activation(warm2[:], warm[:], AF.Sqrt)

            # center rows first: their norm computes while banks stream
            ctile = gath.tile([P, D], bf16, tag="ctile")
            nc.gpsimd.indirect_dma_start(
                out=ctile[:],
                out_offset=None,
                in_=z_t[:],
                in_offset=bass.IndirectOffsetOnAxis(ap=cidx_sb[:, 0:1], axis=0),
            )

            # neighbor banks: one big dma_gather per 32768-row slab
            G = []
            off16 = 0
            for b in range(NBANKS):
                nb = CAPS[b] // P
                g = gath.tile([P, nb, D], bf16, tag=f"g{b}")
                hi = min((b + 1) * BANK, N)
                nc.gpsimd.dma_gather(
                    out_ap=g[:],
                    in_ap=z_t[b * BANK:hi, :],
                    idxs_ap=idx16_sb[:, off16:off16 + CAPS[b] // 16],
                    num_idxs=CAPS[b],
                    num_idxs_reg=CAPS[b],
                    elem_size=D,
                    queue_num=b % 4,
                )
                G.append(g)
                off16 += CAPS[b] // 16

            # masks: 8 chunks split across the SP and ACT HWDGE queues so
            # they land within the first ~10us and never stall the PE
            mask_sb = const.tile([P, NBAT * P], bf16, tag="nmask")
            ck = (NBAT * P) // 8
            for ci in range(8):
                lo, hi_ = ci * ck, (ci + 1) * ck if ci < 7 else NBAT * P
                eng = nc.sync if ci % 2 == 0 else nc.scalar
                eng.dma_start(mask_sb[:, lo:hi_], mask_t[:, lo:hi_])
            cmask_sb = const.tile([P, 1], f32, tag="cmask")
            nc.sync.dma_start(cmask_sb[:], cmask_t[:])

            # center norm (early)
            ssq_c = const.tile([P, 1], f32, tag="ssqc")
            sqr_c = const.tile([P, 1], f32, tag="sqrc")
            rno_c = const.tile([P, 1], f32, tag="rnoc")
            sc0 = scr.tile([P, D], bf16, tag="dve_sq")
            nc.vector.scalar_tensor_tensor(
                out=sc0[:], in0=ctile[:], scalar=1.0, in1=ctile[:],
                op0=ALU.mult, op1=ALU.mult, accum_out=ssq_c[:],
            )
            nc.scalar.activation(sqr_c[:], ssq_c[:], AF.Sqrt)
            nc.vector.reciprocal(rno_c[:], sqr_c[:])

            V = ps.tile([P, D], f32, tag="V")
            ssq = const.tile([P, NBAT], f32, tag="ssq")
            sqr = const.tile([P, NBAT], f32, tag="sqr")
            rno = const.tile([P, NBAT], f32, tag="rno")

            batches = []           # (bank, col) per global batch
            for b in range(NBANKS):
                for j in range(CAPS[b] // P):
                    batches.append((b, j))
            assert len(batches) == NBAT

            # per group of 8 batches: ssq split ~1/3 ACT, 2/3 DVE; sqrt on
            # ACT + recip on DVE; mask*rnorm split ACT/DVE for early batches
            # and GpSimd (idle after desc-gen) for late ones; matmul on PE
            for g0 in range(0, NBAT, GRP):
                g1 = min(g0 + GRP, NBAT)
                for t in range(g0, g1):
                    b, j = batches[t]
                    src = G[b][:, j, :]
                    if t % 3 == 0:
                        sc = scr.tile([P, D], bf16, tag="act_sq")
                        nc.scalar.activation(
                            sc[:], src, AF.Square, accum_out=ssq[:, t:t + 1]
                        )
                    else:
                        sc = scr.tile([P, D], bf16, tag="dve_sq")
                        nc.vector.scalar_tensor_tensor(
                            out=sc[:], in0=src, scalar=1.0, in1=src,
                            op0=ALU.mult, op1=ALU.mult,
                            accum_out=ssq[:, t:t + 1],
                        )
                nc.scalar.activation(sqr[:, g0:g1], ssq[:, g0:g1], AF.Sqrt)
                nc.vector.reciprocal(rno[:, g0:g1], sqr[:, g0:g1])
                # pass 2 for this group: mask*rnorm then PE accumulate
                for t in range(g0, g1):
                    b, j = batches[t]
                    src = G[b][:, j, :]
                    m = mrn.tile([P, P], bf16, tag="mrn")
                    msl = mask_sb[:, t * P:(t + 1) * P]
                    if t >= 26:
                        nc.gpsimd.tensor_scalar_mul(m[:], msl, rno[:, t:t + 1])
                    elif t % 2 == 0:
                        nc.scalar.activation(
                            m[:], msl, AF.Copy, scale=rno[:, t:t + 1]
                        )
                    else:
                        nc.vector.tensor_scalar_mul(m[:], msl, rno[:, t:t + 1])
                    nc.tensor.matmul(
                        out=V[:], lhsT=m[:], rhs=src,
                        start=(t == 0), stop=(t == NBAT - 1),
                    )

            wscr = scr.tile([P, D], f32, tag="wscr")
            r = const.tile([P, 1], f32, tag="r")
            nc.vector.scalar_tensor_tensor(
                out=wscr[:], in0=ctile[:], scalar=rno_c[:, :1], in1=V[:],
                op0=ALU.mult, op1=ALU.mult, accum_out=r[:],
            )

            res_ps = ps.tile([1, 1], f32, tag="res")
            nc.tensor.matmul(
                out=res_ps[:], lhsT=cmask_sb[:], rhs=r[:], start=True, stop=True
            )
            res_sb = const.tile([1, 1], f32, tag="res_sb")
            nc.vector.tensor_copy(res_sb[:], res_ps[:])
            nc.sync.dma_start(out_t[:], res_sb[:])

    nc.compile()
    return nc


def _get_module():
    if "nc" not in _cache:
        _cache["nc"] = _build_module()
    return _cache["nc"]


def _make_in_maps(z, knn_neighbors, sample_indices):
    import ml_dtypes

    bf = ml_dtypes.bfloat16
    z = np.asarray(z, dtype=np.float32)
    knn = np.asarray(knn_neighbors).astype(np.int64)
    sample = np.asarray(sample_indices).astype(np.int64).ravel()
    assert z.shape == (N, D) and knn.shape == (N, K) and sample.shape == (S,)

    z_bf = np.ascontiguousarray(z.astype(bf))

    caps = np.asarray(CAPS)
    bank_off = np.concatenate([[0], np.cumsum(caps)])  # slot offset per bank

    in_maps = []
    for c in range(NCORES):
        s_ids = sample[c * SPC:(c + 1) * SPC]
        rows = knn[s_ids].ravel()                     # [4000] neighbor rows
        cid = np.repeat(np.arange(SPC), K)            # center of each row
        bank = (rows // BANK).astype(np.int64)

        lidx = np.zeros(sum(CAPS), dtype=np.int16)    # bank-local indices
        mask = np.zeros((P, NBAT * P), dtype=bf)
        for b in range(NBANKS):
            sel = np.nonzero(bank == b)[0]
            cnt = len(sel)
            assert cnt <= CAPS[b], f"bank {b} overflow: {cnt} > {CAPS[b]}"
            loc = (rows[sel] - b * BANK).astype(np.int16)
            padval = loc[0] if cnt > 0 else np.int16(0)
            o = bank_off[b]
            lidx[o:o + cnt] = loc
            lidx[o + cnt:o + CAPS[b]] = padval
            gslot = o + np.arange(cnt)
            mask[gslot % P, (gslot // P) * P + cid[sel]] = 1.0

        # wrap into 16 partitions: descriptor i <- idx16[i%16, i//16],
        # per bank; then replicate across the 8 gpsimd Q7 cores.
        idx16 = np.zeros((16, IDXCOLS), dtype=np.int16)
        for b in range(NBANKS):
            o = bank_off[b]
            o16 = o // 16
            li = lidx[o:o + CAPS[b]]
            idx16[:, o16:o16 + CAPS[b] // 16] = li.reshape(CAPS[b] // 16, 16).T
        idx16 = np.tile(idx16, (8, 1))

        s_pad = np.zeros(P, dtype=np.int64)
        s_pad[:SPC] = s_ids
        cidx = s_pad.astype(np.int32).reshape(P, 1)
        cmask = (np.arange(P) < SPC).astype(np.float32).reshape(P, 1)

        in_maps.append({
            "z": z_bf, "idx16": idx16, "nmask": mask,
            "cidx": cidx, "cmask": cmask,
        })
    return in_maps


def _combine(results):
    total = sum(float(res["out"][0, 0]) for res in results)
    return np.array(1.0 - total / (S * K), dtype=np.float32)


def kernel(z, knn_neighbors, sample_indices):
    from concourse.bass_utils import run_bass_kernel_spmd

    nc = _get_module()
    in_maps = _make_in_maps(z, knn_neighbors, sample_indices)
    out = run_bass_kernel_spmd(nc, in_maps, core_ids=list(range(NCORES)))
    return _combine(out.results)


def run_profiled(z, knn_neighbors, sample_indices, **kw):
    """Dev helper: same as kernel() but returns (loss, BassKernelResults)
    with trace/profile enabled."""
    from concourse.bass_utils import run_bass_kernel_spmd

    nc = _get_module()
    in_maps = _make_in_maps(z, knn_neighbors, sample_indices)
    out = run_bass_kernel_spmd(
        nc, in_maps, core_ids=list(range(NCORES)), trace=True, **kw
    )
    return _combine(out.results), out


# revision 7
# speedup vs baseline: 1.2686x; 1.2686x over previous
"""NeighborConsistencyLoss on 8 Trainium2 NeuronCores.

Math:  loss = mean_s(1 - mean_k cos(z[s], z[knn[s,k]]))
            = 1 - (1/(S*K)) * sum_{s,k} u(z[s]) . u(z[knn[s,k]])
where u(x) = x/|x| (eps in max(|a||b|, eps) never binds for randn data).

Sharding: replicate z (pre-cast to bf16 on host), shard the S=1000
sampled centers across 8 cores (125 each, padded to 128).

Per-core device pipeline (v2 — dma_gather based):
 - z lives in HBM as bf16 [N, D] (host-side cast; halves gather traffic
   and feeds the PE directly). The 125*32 = 4000 neighbor rows are
   gathered by 7 `dma_gather` (InstDMAGatherAnt) instructions, one per
   32768-row bank of z (int16 local indices = row - 32768*b). One
   gather instruction carries ~768 descriptors at ~0.34ns/desc of
   SWDGE time vs 994ns+128desc for classic indirect1d — this removes
   the GpSimd descriptor-generation serial bottleneck that dominated
   v1 (33 indirect1d = 48us serial).
 - Banks are capped at 768 slots (6 batches of 128) for banks 0-5 and
   128 for bank 6 (rows 196608..199999); unused slots are padded with
   a duplicate valid index (gathers real data, masked out later) so no
   NaN/garbage can reach the matmuls. Gather layout: descriptor i ->
   dst[i%128, i//128, :].
 - Per 128-row batch: ssq via ACT Square+accum / DVE mult+accum
   (alternating), rnorm = 1/sqrt batched per 4 batches, then
   mask_rn = host_mask * rnorm (one [128,128] op, alternating engines)
   where host_mask[p, s] = 1 iff slot p of the batch belongs to center
   s. The PE accumulates V[s,:] += mask_rn^T @ raw_bf16_rows, folding
   both the neighbor-sum AND the per-row normalization into the
   matmul (no per-row scale pass over [128,512] data).
 - Centers: one classic indirect1d gather (128 rows, int32), norm
   computed early, r[s] = rnorm_c[s] * sum_d c[s,d]*V[s,d] (fused DVE
   op with accum), partial = cmask^T @ r via a tiny matmul.
 - Host combines: loss = 1 - total/(S*K).
"""

import numpy as np

N, D, K, S = 200000, 512, 32, 1000
NCORES = 8
SPC = S // NCORES            # 125 samples per core
P = 128
BANK = 32768                 # int16-addressable rows per dma_gather bank
NBANKS = 7                   # ceil(N / BANK)
CAPS = [768] * 6 + [128]     # slots per bank (multiple of 128)
NBAT = sum(CAPS) // P        # 37 neighbor batches
IDXCOLS = sum(CAPS) // 16    # 296 int16 columns
GRP = 8                      # batches per rnorm group

_cache = {}


def _build_module():
    import concourse.bacc as bacc
    import concourse.bass as bass
    import concourse.mybir as mybir
    import concourse.tile as tile

    f32 = mybir.dt.float32
    bf16 = mybir.dt.bfloat16
    i32 = mybir.dt.int32
    i16 = mybir.dt.int16
    AF = mybir.ActivationFunctionType
    ALU = mybir.AluOpType

    nc = bacc.Bacc(None, target_bir_lowering=False,
                   dynamic_dma_scratch_size=2**16, num_swdge_queues=4)
    z_t = nc.dram_tensor("z", [N, D], bf16, kind="ExternalInput")
    idx16_t = nc.dram_tensor("idx16", [P, IDXCOLS], i16, kind="ExternalInput")
    mask_t = nc.dram_tensor("nmask", [P, NBAT * P], bf16, kind="ExternalInput")
    cidx_t = nc.dram_tensor("cidx", [P, 1], i32, kind="ExternalInput")
    cmask_t = nc.dram_tensor("cmask", [P, 1], f32, kind="ExternalInput")
    out_t = nc.dram_tensor("out", [1, 1], f32, kind="ExternalOutput")

    with tile.TileContext(nc) as tc:
        with (
            tc.tile_pool(name="const", bufs=1) as const,
            tc.tile_pool(name="gath", bufs=1) as gath,
            tc.tile_pool(name="scr", bufs=2) as scr,
            tc.tile_pool(name="mrn", bufs=4) as mrn,
            tc.tile_pool(name="ps", bufs=1, space="PSUM") as ps,
        ):
            cidx_sb = const.tile([P, 1], i32, tag="cidx")
            nc.sync.dma_start(cidx_sb[:], cidx_t[:])
            idx16_sb = const.tile([P, IDXCOLS], i16, tag="idx16")
            nc.sync.dma_start(idx16_sb[:], idx16_t[:])

            # warm both activation tables during the dead startup window so
            # the 1.28us ACT_TABLE_LOADs don't stall the pipeline mid-stream
            warm = const.tile([P, 1], f32, tag="warm")
            warm2 = const.tile([P, 1], f32, tag="warm2")
            nc.vector.memset(warm[:], 1.0)
            nc.scalar.activation(warm2[:], warm[:], AF.Square)
            nc.scalar.activation(warm2[:], warm[:], AF.Sqrt)

            # center rows first: their norm computes while banks stream
            ctile = gath.tile([P, D], bf16, tag="ctile")
            nc.gpsimd.indirect_dma_start(
                out=ctile[:],
                out_offset=None,
                in_=z_t[:],
                in_offset=bass.IndirectOffsetOnAxis(ap=cidx_sb[:, 0:1], axis=0),
            )

            # neighbor banks: one big dma_gather per 32768-row slab
            G = []
            off16 = 0
            for b in range(NBANKS):
                nb = CAPS[b] // P
                g = gath.tile([P, nb, D], bf16, tag=f"g{b}")
                hi = min((b + 1) * BANK, N)
                nc.gpsimd.dma_gather(
                    out_ap=g[:],
                    in_ap=z_t[b * BANK:hi, :],
                    idxs_ap=idx16_sb[:, off16:off16 + CAPS[b] // 16],
                    num_idxs=CAPS[b],
                    num_idxs_reg=CAPS[b],
                    elem_size=D,
                    queue_num=b % 4,
                )
                G.append(g)
                off16 += CAPS[b] // 16

            # masks: 8 chunks split across the SP and ACT HWDGE queues so
            # they land within the first ~10us and never stall the PE
            mask_sb = const.tile([P, NBAT * P], bf16, tag="nmask")
            ck = (NBAT * P) // 8
            for ci in range(8):
                lo, hi_ = ci * ck, (ci + 1) * ck if ci < 7 else NBAT * P
                eng = nc.sync if ci % 2 == 0 else nc.scalar
                eng.dma_start(mask_sb[:, lo:hi_], mask_t[:, lo:hi_])
            cmask_sb = const.tile([P, 1], f32, tag="cmask")
            nc.sync.dma_start(cmask_sb[:], cmask_t[:])

            # center norm (early)
            ssq_c = const.tile([P, 1], f32, tag="ssqc")
            sqr_c = const.tile([P, 1], f32, tag="sqrc")
            rno_c = const.tile([P, 1], f32, tag="rnoc")
            sc0 = scr.tile([P, D], bf16, tag="dve_sq")
            nc.vector.scalar_tensor_tensor(
                out=sc0[:], in0=ctile[:], scalar=1.0, in1=ctile[:],
                op0=ALU.mult, op1=ALU.mult, accum_out=ssq_c[:],
            )
            nc.scalar.activation(sqr_c[:], ssq_c[:], AF.Sqrt)
            nc.vector.reciprocal(rno_c[:], sqr_c[:])

            V = ps.tile([P, D], f32, tag="V")
            ssq = const.tile([P, NBAT], f32, tag="ssq")
            sqr = const.tile([P, NBAT], f32, tag="sqr")
            rno = const.tile([P, NBAT], f32, tag="rno")

            batches = []           # (bank, col) per global batch
            for b in range(NBANKS):
                for j in range(CAPS[b] // P):
                    batches.append((b, j))
            assert len(batches) == NBAT

            # per group of 8 batches: ssq split ~1/3 ACT, 2/3 DVE; sqrt on
            # ACT + recip on DVE; mask*rnorm split ACT/DVE for early batches
            # and GpSimd (idle after desc-gen) for late ones; matmul on PE
            for g0 in range(0, NBAT, GRP):
                g1 = min(g0 + GRP, NBAT)
                for t in range(g0, g1):
                    b, j = batches[t]
                    src = G[b][:, j, :]
                    if t % 7 < 4:
                        sc = scr.tile([P, D], bf16, tag="act_sq")
                        nc.scalar.activation(
                            sc[:], src, AF.Square, accum_out=ssq[:, t:t + 1]
                        )
                    else:
                        sc = scr.tile([P, D], bf16, tag="dve_sq")
                        nc.vector.scalar_tensor_tensor(
                            out=sc[:], in0=src, scalar=1.0, in1=src,
                            op0=ALU.mult, op1=ALU.mult,
                            accum_out=ssq[:, t:t + 1],
                        )
                nc.scalar.activation(sqr[:, g0:g1], ssq[:, g0:g1], AF.Sqrt)
                nc.vector.reciprocal(rno[:, g0:g1], sqr[:, g0:g1])
                # pass 2 for this group: mask*rnorm then PE accumulate
                for t in range(g0, g1):
                    b, j = batches[t]
                    src = G[b][:, j, :]
                    m = mrn.tile([P, P], bf16, tag="mrn")
                    msl = mask_sb[:, t * P:(t + 1) * P]
                    nc.vector.tensor_scalar_mul(m[:], msl, rno[:, t:t + 1])
                    nc.tensor.matmul(
                        out=V[:], lhsT=m[:], rhs=src,
                        start=(t == 0), stop=(t == NBAT - 1),
                    )

            wscr = scr.tile([P, D], f32, tag="wscr")
            r = const.tile([P, 1], f32, tag="r")
            nc.vector.scalar_tensor_tensor(
                out=wscr[:], in0=ctile[:], scalar=rno_c[:, :1], in1=V[:],
                op0=ALU.mult, op1=ALU.mult, accum_out=r[:],
            )

            res_ps = ps.tile([1, 1], f32, tag="res")
            nc.tensor.matmul(
                out=res_ps[:], lhsT=cmask_sb[:], rhs=r[:], start=True, stop=True
            )
            res_sb = const.tile([1, 1], f32, tag="res_sb")
            nc.vector.tensor_copy(res_sb[:], res_ps[:])
            nc.sync.dma_start(out_t[:], res_sb[:])

    nc.compile()
    return nc


def _get_module():
    if "nc" not in _cache:
        _cache["nc"] = _build_module()
    return _cache["nc"]


def _make_in_maps(z, knn_neighbors, sample_indices):
    import ml_dtypes

    bf = ml_dtypes.bfloat16
    z = np.asarray(z, dtype=np.float32)
    knn = np.asarray(knn_neighbors).astype(np.int64)
    sample = np.asarray(sample_indices).astype(np.int64).ravel()
    assert z.shape == (N, D) and knn.shape == (N, K) and sample.shape == (S,)

    z_bf = np.ascontiguousarray(z.astype(bf))

    caps = np.asarray(CAPS)
    bank_off = np.concatenate([[0], np.cumsum(caps)])  # slot offset per bank

    in_maps = []
    for c in range(NCORES):
        s_ids = sample[c * SPC:(c + 1) * SPC]
        rows = knn[s_ids].ravel()                     # [4000] neighbor rows
        cid = np.repeat(np.arange(SPC), K)            # center of each row
        bank = (rows // BANK).astype(np.int64)

        lidx = np.zeros(sum(CAPS), dtype=np.int16)    # bank-local indices
        mask = np.zeros((P, NBAT * P), dtype=bf)
        for b in range(NBANKS):
            sel = np.nonzero(bank == b)[0]
            cnt = len(sel)
            assert cnt <= CAPS[b], f"bank {b} overflow: {cnt} > {CAPS[b]}"
            loc = (rows[sel] - b * BANK).astype(np.int16)
            padval = loc[0] if cnt > 0 else np.int16(0)
            o = bank_off[b]
            lidx[o:o + cnt] = loc
            lidx[o + cnt:o + CAPS[b]] = padval
            gslot = o + np.arange(cnt)
            mask[gslot % P, (gslot // P) * P + cid[sel]] = 1.0

        # wrap into 16 partitions: descriptor i <- idx16[i%16, i//16],
        # per bank; then replicate across the 8 gpsimd Q7 cores.
        idx16 = np.zeros((16, IDXCOLS), dtype=np.int16)
        for b in range(NBANKS):
            o = bank_off[b]
            o16 = o // 16
            li = lidx[o:o + CAPS[b]]
            idx16[:, o16:o16 + CAPS[b] // 16] = li.reshape(CAPS[b] // 16, 16).T
        idx16 = np.tile(idx16, (8, 1))

        s_pad = np.zeros(P, dtype=np.int64)
        s_pad[:SPC] = s_ids
        cidx = s_pad.astype(np.int32).reshape(P, 1)
        cmask = (np.arange(P) < SPC).astype(np.float32).reshape(P, 1)

        in_maps.append({
            "z": z_bf, "idx16": idx16, "nmask": mask,
            "cidx": cidx, "cmask": cmask,
        })
    return in_maps


def _combine(results):
    total = sum(float(res["out"][0, 0]) for res in results)
    return np.array(1.0 - total / (S * K), dtype=np.float32)


def kernel(z, knn_neighbors, sample_indices):
    from concourse.bass_utils import run_bass_kernel_spmd

    nc = _get_module()
    in_maps = _make_in_maps(z, knn_neighbors, sample_indices)
    out = run_bass_kernel_spmd(nc, in_maps, core_ids=list(range(NCORES)))
    return _combine(out.results)


def run_profiled(z, knn_neighbors, sample_indices, **kw):
    """Dev helper: same as kernel() but returns (loss, BassKernelResults)
    with trace/profile enabled."""
    from concourse.bass_utils import run_bass_kernel_spmd

    nc = _get_module()
    in_maps = _make_in_maps(z, knn_neighbors, sample_indices)
    out = run_bass_kernel_spmd(
        nc, in_maps, core_ids=list(range(NCORES)), trace=True, **kw
    )
    return _combine(out.results), out


# revision 8
# speedup vs baseline: 1.2943x; 1.0202x over previous
"""NeighborConsistencyLoss on 8 Trainium2 NeuronCores.

Math:  loss = mean_s(1 - mean_k cos(z[s], z[knn[s,k]]))
            = 1 - (1/(S*K)) * sum_{s,k} u(z[s]) . u(z[knn[s,k]])
where u(x) = x/|x| (eps in max(|a||b|, eps) never binds for randn data).

Sharding: replicate z (pre-cast to bf16 on host), shard the S=1000
sampled centers across 8 cores (125 each, padded to 128).

Per-core device pipeline (v2 — dma_gather based):
 - z lives in HBM as bf16 [N, D] (host-side cast; halves gather traffic
   and feeds the PE directly). The 125*32 = 4000 neighbor rows are
   gathered by 7 `dma_gather` (InstDMAGatherAnt) instructions, one per
   32768-row bank of z (int16 local indices = row - 32768*b). One
   gather instruction carries ~768 descriptors at ~0.34ns/desc of
   SWDGE time vs 994ns+128desc for classic indirect1d — this removes
   the GpSimd descriptor-generation serial bottleneck that dominated
   v1 (33 indirect1d = 48us serial).
 - Banks are capped at 768 slots (6 batches of 128) for banks 0-5 and
   128 for bank 6 (rows 196608..199999); unused slots are padded with
   a duplicate valid index (gathers real data, masked out later) so no
   NaN/garbage can reach the matmuls. Gather layout: descriptor i ->
   dst[i%128, i//128, :].
 - Per 128-row batch: ssq via ACT Square+accum / DVE mult+accum
   (alternating), rnorm = 1/sqrt batched per 4 batches, then
   mask_rn = host_mask * rnorm (one [128,128] op, alternating engines)
   where host_mask[p, s] = 1 iff slot p of the batch belongs to center
   s. The PE accumulates V[s,:] += mask_rn^T @ raw_bf16_rows, folding
   both the neighbor-sum AND the per-row normalization into the
   matmul (no per-row scale pass over [128,512] data).
 - Centers: one classic indirect1d gather (128 rows, int32), norm
   computed early, r[s] = rnorm_c[s] * sum_d c[s,d]*V[s,d] (fused DVE
   op with accum), partial = cmask^T @ r via a tiny matmul.
 - Host combines: loss = 1 - total/(S*K).
"""

import numpy as np

N, D, K, S = 200000, 512, 32, 1000
NCORES = 8
SPC = S // NCORES            # 125 samples per core
P = 128
BANK = 32768                 # int16-addressable rows per dma_gather bank
NBANKS = 7                   # ceil(N / BANK)
CAPS = [768] * 6 + [128]     # slots per bank (multiple of 128)
NBAT = sum(CAPS) // P        # 37 neighbor batches
IDXCOLS = sum(CAPS) // 16    # 296 int16 columns
GRP = 4                      # batches per rnorm group

_cache = {}


def _build_module():
    import concourse.bacc as bacc
    import concourse.bass as bass
    import concourse.mybir as mybir
    import concourse.tile as tile

    f32 = mybir.dt.float32
    bf16 = mybir.dt.bfloat16
    i32 = mybir.dt.int32
    i16 = mybir.dt.int16
    fp8 = mybir.dt.float8e4
    AF = mybir.ActivationFunctionType
    ALU = mybir.AluOpType

    nc = bacc.Bacc(None, target_bir_lowering=False,
                   dynamic_dma_scratch_size=2**16, num_swdge_queues=4)
    z_t = nc.dram_tensor("z", [N, D], bf16, kind="ExternalInput")
    idx16_t = nc.dram_tensor("idx16", [P, IDXCOLS], i16, kind="ExternalInput")
    mask_t = nc.dram_tensor("nmask", [P, NBAT * P], fp8, kind="ExternalInput")
    cidx_t = nc.dram_tensor("cidx", [P, 1], i32, kind="ExternalInput")
    cmask_t = nc.dram_tensor("cmask", [P, 1], f32, kind="ExternalInput")
    out_t = nc.dram_tensor("out", [1, 1], f32, kind="ExternalOutput")

    with tile.TileContext(nc) as tc:
        with (
            tc.tile_pool(name="const", bufs=1) as const,
            tc.tile_pool(name="gath", bufs=1) as gath,
            tc.tile_pool(name="scr", bufs=2) as scr,
            tc.tile_pool(name="mrn", bufs=4) as mrn,
            tc.tile_pool(name="ps", bufs=1, space="PSUM") as ps,
        ):
            cidx_sb = const.tile([P, 1], i32, tag="cidx")
            nc.sync.dma_start(cidx_sb[:], cidx_t[:])
            idx16_sb = const.tile([P, IDXCOLS], i16, tag="idx16")
            nc.sync.dma_start(idx16_sb[:], idx16_t[:])

            # warm both activation tables during the dead startup window so
            # the 1.28us ACT_TABLE_LOADs don't stall the pipeline mid-stream
            warm = const.tile([P, 1], f32, tag="warm")
            warm2 = const.tile([P, 1], f32, tag="warm2")
            nc.vector.memset(warm[:], 1.0)
            nc.scalar.activation(warm2[:], warm[:], AF.Square)
            nc.scalar.activation(warm2[:], warm[:], AF.Sqrt)

            # masks first: 8 chunks split across the SP and ACT HWDGE
            # queues so their transfers complete before the gather drains
            # monopolize the DMA engines
            mask_sb = const.tile([P, NBAT * P], fp8, tag="nmask")
            ck = (NBAT * P) // 8
            for ci in range(8):
                lo, hi_ = ci * ck, (ci + 1) * ck if ci < 7 else NBAT * P
                eng = nc.sync if ci % 2 == 0 else nc.scalar
                eng.dma_start(mask_sb[:, lo:hi_], mask_t[:, lo:hi_])

            # center rows: their norm computes while banks stream
            ctile = gath.tile([P, D], bf16, tag="ctile")
            nc.gpsimd.indirect_dma_start(
                out=ctile[:],
                out_offset=None,
                in_=z_t[:],
                in_offset=bass.IndirectOffsetOnAxis(ap=cidx_sb[:, 0:1], axis=0),
            )

            # neighbor banks: one big dma_gather per 32768-row slab
            G = []
            off16 = 0
            for b in range(NBANKS):
                nb = CAPS[b] // P
                g = gath.tile([P, nb, D], bf16, tag=f"g{b}")
                hi = min((b + 1) * BANK, N)
                nc.gpsimd.dma_gather(
                    out_ap=g[:],
                    in_ap=z_t[b * BANK:hi, :],
                    idxs_ap=idx16_sb[:, off16:off16 + CAPS[b] // 16],
                    num_idxs=CAPS[b],
                    num_idxs_reg=CAPS[b],
                    elem_size=D,
                    queue_num=b % 4,
                )
                G.append(g)
                off16 += CAPS[b] // 16

            cmask_sb = const.tile([P, 1], f32, tag="cmask")
            nc.sync.dma_start(cmask_sb[:], cmask_t[:])

            # center norm (early)
            ssq_c = const.tile([P, 1], f32, tag="ssqc")
            sqr_c = const.tile([P, 1], f32, tag="sqrc")
            rno_c = const.tile([P, 1], f32, tag="rnoc")
            sc0 = scr.tile([P, D], bf16, tag="dve_sq")
            nc.vector.scalar_tensor_tensor(
                out=sc0[:], in0=ctile[:], scalar=1.0, in1=ctile[:],
                op0=ALU.mult, op1=ALU.mult, accum_out=ssq_c[:],
            )
            nc.scalar.activation(sqr_c[:], ssq_c[:], AF.Sqrt)
            nc.vector.reciprocal(rno_c[:], sqr_c[:])

            V = ps.tile([P, D], f32, tag="V")
            ssq = const.tile([P, NBAT], f32, tag="ssq")
            sqr = const.tile([P, NBAT], f32, tag="sqr")
            rno = const.tile([P, NBAT], f32, tag="rno")

            batches = []           # (bank, col) per global batch
            for b in range(NBANKS):
                for j in range(CAPS[b] // P):
                    batches.append((b, j))
            assert len(batches) == NBAT

            # per group of 8 batches: ssq split ~1/3 ACT, 2/3 DVE; sqrt on
            # ACT + recip on DVE; mask*rnorm split ACT/DVE for early batches
            # and GpSimd (idle after desc-gen) for late ones; matmul on PE
            for g0 in range(0, NBAT, GRP):
                g1 = min(g0 + GRP, NBAT)
                for t in range(g0, g1):
                    b, j = batches[t]
                    src = G[b][:, j, :]
                    if t % 7 < 4:
                        sc = scr.tile([P, D], bf16, tag="act_sq")
                        nc.scalar.activation(
                            sc[:], src, AF.Square, accum_out=ssq[:, t:t + 1]
                        )
                    else:
                        sc = scr.tile([P, D], bf16, tag="dve_sq")
                        nc.vector.scalar_tensor_tensor(
                            out=sc[:], in0=src, scalar=1.0, in1=src,
                            op0=ALU.mult, op1=ALU.mult,
                            accum_out=ssq[:, t:t + 1],
                        )
                nc.scalar.activation(sqr[:, g0:g1], ssq[:, g0:g1], AF.Sqrt)
                nc.vector.reciprocal(rno[:, g0:g1], sqr[:, g0:g1])
                # pass 2 for this group: mask*rnorm then PE accumulate
                for t in range(g0, g1):
                    b, j = batches[t]
                    src = G[b][:, j, :]
                    m = mrn.tile([P, P], fp8, tag="mrn")
                    msl = mask_sb[:, t * P:(t + 1) * P]
                    nc.vector.tensor_scalar_mul(m[:], msl, rno[:, t:t + 1])
                    nc.tensor.matmul(
                        out=V[:], lhsT=m[:], rhs=src,
                        start=(t == 0), stop=(t == NBAT - 1),
                    )

            wscr = scr.tile([P, D], f32, tag="wscr")
            r = const.tile([P, 1], f32, tag="r")
            nc.vector.scalar_tensor_tensor(
                out=wscr[:], in0=ctile[:], scalar=rno_c[:, :1], in1=V[:],
                op0=ALU.mult, op1=ALU.mult, accum_out=r[:],
            )

            res_ps = ps.tile([1, 1], f32, tag="res")
            nc.tensor.matmul(
                out=res_ps[:], lhsT=cmask_sb[:], rhs=r[:], start=True, stop=True
            )
            res_sb = const.tile([1, 1], f32, tag="res_sb")
            nc.vector.tensor_copy(res_sb[:], res_ps[:])
            nc.sync.dma_start(out_t[:], res_sb[:])

    nc.compile()
    return nc


def _get_module():
    if "nc" not in _cache:
        _cache["nc"] = _build_module()
    return _cache["nc"]


def _make_in_maps(z, knn_neighbors, sample_indices):
    import ml_dtypes

    bf = ml_dtypes.bfloat16
    z = np.asarray(z, dtype=np.float32)
    knn = np.asarray(knn_neighbors).astype(np.int64)
    sample = np.asarray(sample_indices).astype(np.int64).ravel()
    assert z.shape == (N, D) and knn.shape == (N, K) and sample.shape == (S,)

    z_bf = np.ascontiguousarray(z.astype(bf))

    caps = np.asarray(CAPS)
    bank_off = np.concatenate([[0], np.cumsum(caps)])  # slot offset per bank

    in_maps = []
    for c in range(NCORES):
        s_ids = sample[c * SPC:(c + 1) * SPC]
        rows = knn[s_ids].ravel()                     # [4000] neighbor rows
        cid = np.repeat(np.arange(SPC), K)            # center of each row
        bank = (rows // BANK).astype(np.int64)

        lidx = np.zeros(sum(CAPS), dtype=np.int16)    # bank-local indices
        mask = np.zeros((P, NBAT * P), dtype=ml_dtypes.float8_e4m3)
        for b in range(NBANKS):
            sel = np.nonzero(bank == b)[0]
            cnt = len(sel)
            assert cnt <= CAPS[b], f"bank {b} overflow: {cnt} > {CAPS[b]}"
            loc = (rows[sel] - b * BANK).astype(np.int16)
            padval = loc[0] if cnt > 0 else np.int16(0)
            o = bank_off[b]
            lidx[o:o + cnt] = loc
            lidx[o + cnt:o + CAPS[b]] = padval
            gslot = o + np.arange(cnt)
            mask[gslot % P, (gslot // P) * P + cid[sel]] = 1.0

        # wrap into 16 partitions: descriptor i <- idx16[i%16, i//16],
        # per bank; then replicate across the 8 gpsimd Q7 cores.
        idx16 = np.zeros((16, IDXCOLS), dtype=np.int16)
        for b in range(NBANKS):
            o = bank_off[b]
            o16 = o // 16
            li = lidx[o:o + CAPS[b]]
            idx16[:, o16:o16 + CAPS[b] // 16] = li.reshape(CAPS[b] // 16, 16).T
        idx16 = np.tile(idx16, (8, 1))

        s_pad = np.zeros(P, dtype=np.int64)
        s_pad[:SPC] = s_ids
        cidx = s_pad.astype(np.int32).reshape(P, 1)
        cmask = (np.arange(P) < SPC).astype(np.float32).reshape(P, 1)

        in_maps.append({
            "z": z_bf, "idx16": idx16, "nmask": mask,
            "cidx": cidx, "cmask": cmask,
        })
    return in_maps


def _combine(results):
    total = sum(float(res["out"][0, 0]) for res in results)
    return np.array(1.0 - total / (S * K), dtype=np.float32)


def kernel(z, knn_neighbors, sample_indices):
    from concourse.bass_utils import run_bass_kernel_spmd

    nc = _get_module()
    in_maps = _make_in_maps(z, knn_neighbors, sample_indices)
    out = run_bass_kernel_spmd(nc, in_maps, core_ids=list(range(NCORES)))
    return _combine(out.results)


def run_profiled(z, knn_neighbors, sample_indices, **kw):
    """Dev helper: same as kernel() but returns (loss, BassKernelResults)
    with trace/profile enabled."""
    from concourse.bass_utils import run_bass_kernel_spmd

    nc = _get_module()
    in_maps = _make_in_maps(z, knn_neighbors, sample_indices)
    out = run_bass_kernel_spmd(
        nc, in_maps, core_ids=list(range(NCORES)), trace=True, **kw
    )
    return _combine(out.results), out
